# revision 4
# baseline (speedup 1.0000x reference)
"""Trainium2 Bass kernel for the QA-head top-k span-masking problem.

Computation (per batch b):
    logits = seq_hiddens[b] @ W_qa + b_qa          # (S, 2)
    masked = logits * m + (1 - m) * (-1e30)        # ans_mask
    start, end = masked[:, 0], masked[:, 1]
    span[i, j] = start[i] + end[j]  valid iff (i >= 4 and 0 <= j - i < 30)
                                     or (i == j in {1, 2, 3})
    top-5 spans by score (descending), flat index i * S + j

Sharding: pure data parallel, batch b -> NeuronCore b (B == 8 == n_cores).
seq_hiddens is staged pre-transposed ([H, S]) per core so the DMA stream
is contiguous per partition (f32 transposed DMA would generate 4-byte
descriptors). The kernel streams x^T once (16.8 MB/core, the memory
roofline), computes the projection on TensorE in float32r (1 cycle/row
vs fp32's 4 — fp32 PE time would exceed the DMA roofline), applies
bias on ScalarE and the ans_mask on GpSimd, reshapes start/end into a
[128, 32] layout (i = 32p + f), builds the 30 shifted-diagonal candidate
bands C[p, 32d + f] = s[i] + e[i + d] on VectorE, and uses the hardware
per-partition top-8 (max / max_index). The host reduces the 128x8
per-partition maxima to the global top-5, re-scoring the 1024 candidate
spans in exact fp32 so the returned indices match the fp32 reference
despite the reduced-precision device matmul.
"""

import numpy as np

try:
    import concourse.bass as bass  # noqa: F401
except ImportError:  # pragma: no cover - container staging path
    import sys

    sys.path.insert(0, "/opt/trn_rl_repo")

import concourse.bass as bass
import concourse.tile as tile
from concourse import bacc, mybir
from concourse.bass_utils import run_bass_kernel_spmd

B, S, H = 8, 4096, 1024
N_CORES = 8
SBLK = 512              # s-range per PSUM accumulation block
NSB = S // SBLK         # 8 s-blocks
KC = H // 128           # 8 contraction chunks
MAXLEN = 30             # spans have 0 <= j - i < 30
NEG = -1.0e30
F32 = mybir.dt.float32
F32R = mybir.dt.float32r
U32 = mybir.dt.uint32

_CACHE = {}


def _build():
    nc = bacc.Bacc("TRN2", target_bir_lowering=False, debug=False,
                   num_devices=N_CORES)
    xt = nc.dram_tensor("xt", [H, S], F32R, kind="ExternalInput").ap()
    w = nc.dram_tensor("w", [H, 2], F32R, kind="ExternalInput").ap()
    bq = nc.dram_tensor("bq", [2, 1], F32, kind="ExternalInput").ap()
    am = nc.dram_tensor("am", [2, S], F32, kind="ExternalInput").ap()
    out_logits = nc.dram_tensor("out_logits", [2, S], F32,
                                kind="ExternalOutput").ap()
    out_m8 = nc.dram_tensor("out_m8", [128, 8], F32, kind="ExternalOutput").ap()
    out_i8 = nc.dram_tensor("out_i8", [128, 8], U32, kind="ExternalOutput").ap()

    with tile.TileContext(nc) as tc:
        with (
            tc.tile_pool(name="const", bufs=1) as cpool,
            tc.tile_pool(name="xin", bufs=4) as xpool,
            tc.tile_pool(name="blk", bufs=2) as bpool,
            tc.tile_pool(name="psum", bufs=2, space="PSUM") as ppool,
            tc.tile_pool(name="work", bufs=1) as wpool,
        ):
            w_sb = cpool.tile([128, 2 * KC], F32R)
            for kc in range(KC):
                nc.sync.dma_start(w_sb[:, 2 * kc:2 * kc + 2],
                                  w[128 * kc:128 * (kc + 1), :])
            b_sb = cpool.tile([2, 1], F32)
            nc.sync.dma_start(b_sb[:], bq[:])
            am_sb = cpool.tile([2, S], F32)
            nc.sync.dma_start(am_sb[:], am[:])

            logits_sb = wpool.tile([2, S], F32)

            for sb in range(NSB):
                xtile = xpool.tile([128, KC, SBLK], F32R)
                src = xt[:, SBLK * sb:SBLK * (sb + 1)]
                nc.sync.dma_start(xtile[:], src.rearrange("(k p) s -> p k s",
                                                          p=128))
                pt = ppool.tile([2, SBLK], F32)
                for kc in range(KC):
                    nc.tensor.matmul(pt[:],
                                     w_sb[:, 2 * kc:2 * kc + 2],
                                     xtile[:, kc, :],
                                     start=(kc == 0), stop=(kc == KC - 1))
                seg = slice(SBLK * sb, SBLK * (sb + 1))
                t_bias = bpool.tile([2, SBLK], F32, tag="tbias")
                nc.scalar.add(t_bias[:], pt[:], b_sb[:, 0:1])
                # masked = (x+b)*m + (m*1e30 - 1e30), on the otherwise-idle
                # gpsimd engine so VectorE stays free for the top-k stage
                t_nm = bpool.tile([2, SBLK], F32, tag="tnm")
                nc.gpsimd.tensor_scalar(t_nm[:], am_sb[:, seg], 1.0e30, NEG,
                                        mybir.AluOpType.mult,
                                        mybir.AluOpType.add)
                t_p = bpool.tile([2, SBLK], F32, tag="tp")
                nc.gpsimd.tensor_mul(t_p[:], t_bias[:], am_sb[:, seg])
                nc.gpsimd.tensor_add(logits_sb[:, seg], t_p[:], t_nm[:])

            nc.sync.dma_start(out_logits[:], logits_sb[:])

            # Reshape start/end rows into [128, 32] (i = 32p + f) tiles.
            s4 = wpool.tile([128, 32], F32)
            nc.sync.dma_start(s4[:], logits_sb[0:1, :])
            e_ext = wpool.tile([128, 64], F32)
            nc.sync.dma_start(e_ext[:, 0:32], logits_sb[1:2, :])
            # e_ext[p, 32+t] = e[32(p+1) + t] for t < 30 (next-partition
            # spill); row 127 keeps NEG so spans with j >= S stay invalid.
            nc.vector.memset(e_ext[:, 32:32 + MAXLEN], NEG)
            shift_src = logits_sb[1:2, 32:S].rearrange("a (p t) -> a p t",
                                                       t=32)[:, :, 0:MAXLEN]
            nc.sync.dma_start(e_ext[0:127, 32:32 + MAXLEN], shift_src)
            # start positions 0..3 are invalid for every d >= 1 and for (0,0)
            nc.vector.memset(s4[0:1, 0:4], NEG)

            cand = wpool.tile([128, 32 * MAXLEN], F32)
            for d in range(MAXLEN):
                nc.vector.tensor_add(cand[:, 32 * d:32 * d + 32], s4[:],
                                     e_ext[:, d:d + 32])
            # special diagonal cells (1,1), (2,2), (3,3) are valid at d = 0
            nc.vector.tensor_add(cand[0:1, 1:4], logits_sb[0:1, 1:4],
                                 e_ext[0:1, 1:4])

            m8 = wpool.tile([128, 8], F32)
            i8 = wpool.tile([128, 8], U32)
            nc.vector.max(m8[:], cand[:])
            nc.vector.max_index(i8[:], m8[:], cand[:])
            nc.sync.dma_start(out_m8[:], m8[:])
            nc.sync.dma_start(out_i8[:], i8[:])

    nc.compile()
    return nc


def _get_nc():
    if "nc" not in _CACHE:
        _CACHE["nc"] = _build()
    return _CACHE["nc"]


def run_device(seq_hiddens, ans_mask, W_qa, b_qa, trace=False, **kw):
    nc = _get_nc()
    seq_hiddens = np.asarray(seq_hiddens, dtype=np.float32)
    ans_mask = np.asarray(ans_mask, dtype=np.float32)
    w = np.ascontiguousarray(np.asarray(W_qa, dtype=np.float32))
    bq = np.ascontiguousarray(np.asarray(b_qa, dtype=np.float32).reshape(2, 1))
    in_maps = []
    for b in range(N_CORES):
        in_maps.append({
            "xt": np.ascontiguousarray(seq_hiddens[b].T),
            "w": w,
            "bq": bq,
            "am": np.ascontiguousarray(
                np.broadcast_to(ans_mask[b][None, :], (2, S))),
        })
    return run_bass_kernel_spmd(nc, in_maps, core_ids=list(range(N_CORES)),
                                trace=trace, **kw)


def kernel(seq_hiddens, ans_mask, W_qa, b_qa, top_k):
    k = int(top_k)
    assert k <= 8
    seq_hiddens = np.asarray(seq_hiddens, dtype=np.float32)
    ans_mask = np.asarray(ans_mask, dtype=np.float32)
    w = np.asarray(W_qa, dtype=np.float32)
    bq = np.asarray(b_qa, dtype=np.float32).reshape(2)
    res = run_device(seq_hiddens, ans_mask, w, bq)
    start_logits = np.empty((B, S), np.float32)
    end_logits = np.empty((B, S), np.float32)
    top_start = np.empty((B, k), np.int32)
    top_end = np.empty((B, k), np.int32)
    for b in range(B):
        out = res.results[b]
        start_logits[b] = out["out_logits"][0]
        end_logits[b] = out["out_logits"][1]
        # Decode the 1024 device-selected candidate spans, then re-score
        # them in exact fp32 (device matmul ran in float32r).
        q = out["out_i8"].astype(np.int64).ravel()            # [1024]
        p = np.arange(128, dtype=np.int64).repeat(8)
        d, f = q // 32, q % 32
        ii = 32 * p + f
        jj = ii + d
        x = seq_hiddens[b]
        m = ans_mask[b]
        s_exact = (x[ii] @ w[:, 0] + bq[0]) * m[ii] + (1.0 - m[ii]) * NEG
        e_exact = (x[jj] @ w[:, 1] + bq[1]) * m[jj] + (1.0 - m[jj]) * NEG
        score = s_exact.astype(np.float64) + e_exact.astype(np.float64)
        flat = ii * S + jj
        order = np.lexsort((flat, -score))[:k]
        top_start[b] = ii[order].astype(np.int32)
        top_end[b] = jj[order].astype(np.int32)
    return start_logits, end_logits, top_start, top_end
